# revision 21
# baseline (speedup 1.0000x reference)
"""Multi-head attention (B=4, S=2048, D=2048, H=16) on 8 trn2 NeuronCores.

Sharding: tensor-parallel over heads — 2 heads per core. Each core computes
its heads' Q/K/V projections, full attention for those heads, and a partial
output projection (its 256 rows of wo). The host sums the 8 partial outputs.

Precision strategy: fp16 end to end (x, weights, q/k/v, exp, avt, wo all
fp16; PSUM accumulation fp32). fp16 streams at the same 1 col/cycle as
fp32r, but the stationary-operand loads get FWL (fast weight load — 2
elements per 32-bit read, compiler-automatic for non-fp32 dtypes), which
fp32r cannot use; that removes the serialized 4-byte weight-load overhead
that dominated the fp32r baseline's non-stream PE time. fp8 DoubleRow was
tried and rejected: e4m3's ~3.6% element noise propagates at full strength
through the softmax-weighted mean (no sqrt-N averaging) and blows the 2e-2
budget. fp16 keeps the error at ~1e-3. DMA also halves (x ships fp16, out
partials return fp16).

Engine budget: ACT does only the exps (the critical 33.5M-element
transcendental load); q/k/v PSUM->SBUF conversions are fused scale+bias
tensor_scalar ops on DVE; out-proj PSUM drains alternate DVE/ACT.
"""
import os
import sys

sys.path.insert(0, "/opt/trn_rl_repo")
import numpy as np

B, S, D, H = 4, 2048, 2048, 16
HD = 128
NCORES = 8
HP = H // NCORES          # heads per core = 2
DC = HP * HD              # per-core slice of D = 256
TOK = B * S               # 8192
SCALE = HD ** -0.5
NDC = D // 128            # 16 contraction chunks for the projections
SPAN = 256                # token span per projection step
NSPAN = S // SPAN         # 8 spans per batch
QS = 512                  # query span in attention
NQS = S // QS             # 4
NKC = S // 128            # 16 key chunks

LAST_EXEC_NS = None
_BUILT = None


def _chunk128(w: np.ndarray) -> np.ndarray:
    """[D, N] -> [128, NDC, N]: contraction row 128*c + p."""
    n = w.shape[1]
    return np.ascontiguousarray(
        w.reshape(NDC, 128, n).transpose(1, 0, 2))


def _build():
    global _BUILT
    if _BUILT is not None:
        return _BUILT
    import concourse.tile as tile
    from concourse import bacc, mybir

    F16 = mybir.dt.float16
    F32 = mybir.dt.float32
    Exp = mybir.ActivationFunctionType.Exp
    Mult = mybir.AluOpType.mult
    Add = mybir.AluOpType.add

    nc = bacc.Bacc("TRN2", target_bir_lowering=False, debug=False)
    xt = nc.dram_tensor("xt", [128, NDC, TOK], F16, kind="ExternalInput")
    wq = nc.dram_tensor("wq", [128, NDC, DC], F16, kind="ExternalInput")
    wk = nc.dram_tensor("wk", [128, NDC, DC], F16, kind="ExternalInput")
    wv = nc.dram_tensor("wv", [128, NDC, DC], F16, kind="ExternalInput")
    wo = nc.dram_tensor("wo", [DC, D], F16, kind="ExternalInput")
    bq2 = nc.dram_tensor("bq2", [HD, HP], F32, kind="ExternalInput")
    bk2 = nc.dram_tensor("bk2", [HD, HP], F32, kind="ExternalInput")
    out = nc.dram_tensor("out", [TOK, D], F16, kind="ExternalOutput")

    with tile.TileContext(nc) as tc:
        with tc.tile_pool(name="const", bufs=1) as cpool, \
             tc.tile_pool(name="xp", bufs=3) as xpool, \
             tc.tile_pool(name="bt", bufs=1) as bpool, \
             tc.tile_pool(name="at", bufs=3) as apool, \
             tc.tile_pool(name="ot", bufs=2) as opool, \
             tc.tile_pool(name="ps", bufs=1, space="PSUM") as ps:

            wq_sb = cpool.tile([128, NDC, DC], F16)
            wk_sb = cpool.tile([128, NDC, DC], F16)
            wv_sb = cpool.tile([128, NDC, DC], F16)
            wo_sb = cpool.tile([128, HP, D], F16)
            ones_sb = cpool.tile([128, 128], F16)
            ebias_sb = cpool.tile([128, 1], F32)
            bq_sb = cpool.tile([HD, HP], F32)
            bk_sb = cpool.tile([HD, HP], F32)
            # wq arrives in two halves so the first Q-projection chunks can
            # start after 512KB instead of 1MB
            nc.sync.dma_start(out=wq_sb[:, 0:NDC // 2, :],
                              in_=wq[:, 0:NDC // 2, :])
            nc.sync.dma_start(out=wq_sb[:, NDC // 2:NDC, :],
                              in_=wq[:, NDC // 2:NDC, :])
            nc.sync.dma_start(out=bq_sb, in_=bq2[:, :])
            nc.sync.dma_start(out=bk_sb, in_=bk2[:, :])
            nc.vector.memset(ones_sb, 1.0)
            nc.vector.memset(ebias_sb, 0.0)

            prefetched = {}
            pending_outproj = None
            for b in range(B):
                # ---- A) Q/K/V projections for batch b ----
                qt_b = bpool.tile([128, HP, S], F16, name="qt_b", tag="qt_b")
                kt_b = bpool.tile([128, HP, S], F16, name="kt_b", tag="kt_b")
                v_b = bpool.tile([128, NKC, DC], F16, name="v_b", tag="v_b")
                for sp in range(NSPAN):
                    t0 = b * S + sp * SPAN
                    if (b, sp) in prefetched:
                        xsp = prefetched.pop((b, sp))
                    else:
                        xsp = xpool.tile([128, NDC, SPAN], F16, name="xsp",
                                         tag="xsp")
                        if b == 0 and sp == 0:
                            # first span arrives in chunk quarters so the
                            # very first matmuls are not gated on the full
                            # 1MB span
                            for c4 in range(0, NDC, 4):
                                nc.sync.dma_start(
                                    out=xsp[:, c4:c4 + 4, :],
                                    in_=xt[:, c4:c4 + 4, t0:t0 + SPAN])
                        else:
                            nc.sync.dma_start(out=xsp,
                                              in_=xt[:, :, t0:t0 + SPAN])
                    if b == 0 and sp == 0:
                        # wk/wv queue behind wq + the first x span so the PE
                        # can start the Q projection as early as possible
                        nc.sync.dma_start(out=wk_sb, in_=wk[:, :, :])
                        nc.sync.dma_start(out=wv_sb, in_=wv[:, :, :])
                    if sp == 1 and pending_outproj is not None:
                        # previous batch's final out-projection, deferred into
                        # this batch's projection stream so its psum drains
                        # don't stall the first projection groups
                        pending_outproj(NQS - 1)
                        pending_outproj = None
                    for h in range(HP):
                        # Q and K accumulate into halves of one PSUM bank
                        qkps = ps.tile([128, 2 * SPAN], F32, name="qkps",
                                       tag="pj", bufs=2)
                        for c in range(NDC):
                            nc.tensor.matmul(
                                qkps[:, 0:SPAN],
                                wq_sb[:, c, h * HD:(h + 1) * HD],
                                xsp[:, c, :],
                                start=(c == 0), stop=(c == NDC - 1))
                        for c in range(NDC):
                            nc.tensor.matmul(
                                qkps[:, SPAN:2 * SPAN],
                                wk_sb[:, c, h * HD:(h + 1) * HD],
                                xsp[:, c, :],
                                start=(c == 0), stop=(c == NDC - 1))
                        nc.vector.tensor_scalar(
                            qt_b[:, h, sp * SPAN:(sp + 1) * SPAN],
                            qkps[:, 0:SPAN], 1.0, bq_sb[:, h:h + 1],
                            Mult, Add)
                        nc.vector.tensor_scalar(
                            kt_b[:, h, sp * SPAN:(sp + 1) * SPAN],
                            qkps[:, SPAN:2 * SPAN], 1.0,
                            bk_sb[:, h:h + 1], Mult, Add)
                    # both V token-chunks accumulate into one PSUM bank
                    vps = ps.tile([128, 2 * DC], F32, name="vps", tag="pj",
                                  bufs=2)
                    for tch in range(SPAN // 128):
                        for c in range(NDC):
                            nc.tensor.matmul(
                                vps[:, tch * DC:(tch + 1) * DC],
                                xsp[:, c, tch * 128:(tch + 1) * 128],
                                wv_sb[:, c, :],
                                start=(c == 0), stop=(c == NDC - 1))
                    for tch in range(SPAN // 128):
                        nc.vector.tensor_copy(
                            v_b[:, sp * (SPAN // 128) + tch, :],
                            vps[:, tch * DC:(tch + 1) * DC])

                if b == 0:
                    # deferred so batch-0 x spans win the DMA queue at startup
                    nc.sync.dma_start(
                        out=wo_sb, in_=wo.rearrange("(c p) n -> p c n", p=128))
                if b + 1 < B:
                    # prefetch the next batch's first three x spans now,
                    # ahead of this batch's 8MB of output DMAs in the ring,
                    # so the next projections don't stall at the batch
                    # boundary
                    for psp in range(3):
                        pt0 = (b + 1) * S + psp * SPAN
                        pxsp = xpool.tile([128, NDC, SPAN], F16, name="xsp",
                                          tag="xsp")
                        nc.sync.dma_start(out=pxsp,
                                          in_=xt[:, :, pt0:pt0 + SPAN])
                        prefetched[(b + 1, psp)] = pxsp

                # ---- B) attention + interleaved partial out-projection ----
                avt_b = bpool.tile([128, HP, S], F16, name="avt_b",
                                   tag="avt_b")

                def emit_outproj(qs, b=b, avt_b=avt_b):
                    # partial out-projection for query span qs; deferred
                    # until the next span's first head has issued so the
                    # avt(h1) normalize sits well behind ~10us of PE work.
                    # (b/avt_b bound at def time: the final span's call runs
                    # inside the NEXT batch's projection stream)
                    for tloc in range(QS // 128):
                        tch = qs * (QS // 128) + tloc
                        out_sb = opool.tile([128, D], F16, name="out_sb",
                                            tag="out_sb")
                        for dsp in range(D // 512):
                            ops = ps.tile([128, 512], F32, name="ops",
                                          tag="pj", bufs=2)
                            for h in range(HP):
                                nc.tensor.matmul(
                                    ops,
                                    avt_b[:, h, tch * 128:(tch + 1) * 128],
                                    wo_sb[:, h, dsp * 512:(dsp + 1) * 512],
                                    start=(h == 0), stop=(h == HP - 1))
                            # split the PSUM drain across DVE and ACT so
                            # neither engine gates the PE
                            if dsp % 2 == 0:
                                nc.vector.tensor_copy(
                                    out_sb[:, dsp * 512:(dsp + 1) * 512], ops)
                            else:
                                nc.scalar.copy(
                                    out_sb[:, dsp * 512:(dsp + 1) * 512], ops)
                            if b == B - 1 and tch == S // 128 - 1:
                                # last tile: drain per 512-col slice so the
                                # final DMA isn't serialized behind all four
                                # copies
                                nc.sync.dma_start(
                                    out=out[b * S + tch * 128:
                                            b * S + (tch + 1) * 128,
                                            dsp * 512:(dsp + 1) * 512],
                                    in_=out_sb[:, dsp * 512:(dsp + 1) * 512])
                        if not (b == B - 1 and tch == S // 128 - 1):
                            nc.sync.dma_start(
                                out=out[b * S + tch * 128:
                                        b * S + (tch + 1) * 128, :],
                                in_=out_sb)

                for qs in range(NQS):
                    for h in range(HP):
                        q_sl = qt_b[:, h, qs * QS:(qs + 1) * QS]
                        av_ps = ps.tile([HD, QS], F32, name="av_ps",
                                        tag="acc", bufs=2)
                        dn_ps = ps.tile([128, QS], F32, name="dn_ps",
                                        tag="acc", bufs=2)

                        def emit_av(kp, p_prev):
                            # AV and the softmax-denominator ones-matmul both
                            # consume the exp tile on the PE — keeps the PE
                            # dense (no DVE/GPSIMD reduction chains). dn goes
                            # first so its stop lands earlier and the DVE
                            # reciprocal overlaps the AV tail.
                            for j in range(2):
                                kc = 2 * kp + j
                                nc.tensor.matmul(
                                    dn_ps, ones_sb, p_prev[:, j, :],
                                    start=(kc == 0), stop=(kc == NKC - 1))
                            for j in range(2):
                                kc = 2 * kp + j
                                nc.tensor.matmul(
                                    av_ps, v_b[:, kc, h * HD:(h + 1) * HD],
                                    p_prev[:, j, :],
                                    start=(kc == 0), stop=(kc == NKC - 1))

                        p_prev = None
                        for kp in range(NKC // 2):
                            # two key-chunks share one psum tile and one exp;
                            # AV of pair kp-1 is emitted after the scores of
                            # pair kp so the PE never heads-of-line blocks on
                            # the exp it needs
                            s_ps = ps.tile([128, 2, QS], F32, name="s_ps",
                                           tag="s", bufs=2)
                            p_sb = apool.tile([128, 2, QS], F16, name="p_sb",
                                              tag="p", bufs=3)
                            for j in range(2):
                                kc = 2 * kp + j
                                nc.tensor.matmul(
                                    s_ps[:, j, :],
                                    kt_b[:, h, kc * 128:(kc + 1) * 128], q_sl,
                                    start=True, stop=True)
                            nc.scalar.activation(
                                p_sb, s_ps, Exp, scale=SCALE,
                                bias=ebias_sb[:, 0:1])
                            if p_prev is not None:
                                emit_av(kp - 1, p_prev)
                            p_prev = p_sb
                        emit_av(NKC // 2 - 1, p_prev)
                        recip = apool.tile([128, QS], F32, name="recip",
                                           tag="recip", bufs=1)
                        nc.vector.reciprocal_approx_fast(recip, dn_ps)
                        nc.vector.tensor_mul(
                            avt_b[:, h, qs * QS:(qs + 1) * QS], av_ps, recip)
                        if h == 0 and qs > 0:
                            emit_outproj(qs - 1)
                if b + 1 < B:
                    pending_outproj = emit_outproj
                else:
                    emit_outproj(NQS - 1)
    nc.compile()
    _BUILT = nc
    return nc


def _install_trace_hooks():
    import types
    try:
        import antenv.axon_hooks  # noqa: F401
        return True
    except ImportError:
        pass
    try:
        from trn_agent_boot.trn_boot import _ntff_profile_via_ctypes
        hook = _ntff_profile_via_ctypes('/opt/axon/libaxon_pjrt.so')
        if hook is None:
            return False
        m = types.ModuleType('antenv.axon_hooks')
        m.get_axon_ntff_profile_hook = lambda: hook
        sys.modules['antenv.axon_hooks'] = m
        from concourse import bass_utils
        bass_utils.upload_artifacts = lambda tmpdir: "local://" + tmpdir
        return True
    except Exception:
        return False


def kernel(x, wq, bq, wk, bk, wv, bv, wo, bo):
    global LAST_EXEC_NS
    from concourse.bass_utils import run_bass_kernel_spmd

    x = np.asarray(x, dtype=np.float32)
    wq = np.asarray(wq, dtype=np.float32)
    bq = np.asarray(bq, dtype=np.float32)
    wk = np.asarray(wk, dtype=np.float32)
    bk = np.asarray(bk, dtype=np.float32)
    wv = np.asarray(wv, dtype=np.float32)
    bv = np.asarray(bv, dtype=np.float32)
    wo = np.asarray(wo, dtype=np.float32)
    bo = np.asarray(bo, dtype=np.float32)

    xt16 = _chunk128(x.reshape(TOK, D).T.astype(np.float16))
    in_maps = []
    for i in range(NCORES):
        sl = slice(i * DC, (i + 1) * DC)
        in_maps.append({
            "xt": xt16,
            "wq": _chunk128(wq[:, sl].astype(np.float16)),
            "wk": _chunk128(wk[:, sl].astype(np.float16)),
            "wv": _chunk128(wv[:, sl].astype(np.float16)),
            "wo": wo[sl, :].astype(np.float16),
            "bq2": np.ascontiguousarray(bq[sl].reshape(HP, HD).T),
            "bk2": np.ascontiguousarray(bk[sl].reshape(HP, HD).T),
        })

    trace = bool(os.environ.get("KERNEL_TRACE"))
    if trace:
        trace = _install_trace_hooks()

    nc = _build()
    res = run_bass_kernel_spmd(nc, in_maps, list(range(NCORES)), trace=trace)
    LAST_EXEC_NS = res.exec_time_ns

    total = np.zeros((TOK, D), dtype=np.float32)
    for r in res.results:
        total += r["out"].astype(np.float32)
    # V-bias folds into a constant row: softmax rows sum to 1, so
    # attention(V + 1*bv^T) = attention(V) + 1*bv^T, and (bv @ wo) adds to bo.
    total += bo + bv @ wo
    return total.reshape(B, S, D)


# revision 22
# speedup vs baseline: 1.0003x; 1.0003x over previous
"""Multi-head attention (B=4, S=2048, D=2048, H=16) on 8 trn2 NeuronCores.

Sharding: tensor-parallel over heads — 2 heads per core. Each core computes
its heads' Q/K/V projections, full attention for those heads, and a partial
output projection (its 256 rows of wo). The host sums the 8 partial outputs.

Precision strategy: fp16 end to end (x, weights, q/k/v, exp, avt, wo all
fp16; PSUM accumulation fp32). fp16 streams at the same 1 col/cycle as
fp32r, but the stationary-operand loads get FWL (fast weight load — 2
elements per 32-bit read, compiler-automatic for non-fp32 dtypes), which
fp32r cannot use; that removes the serialized 4-byte weight-load overhead
that dominated the fp32r baseline's non-stream PE time. fp8 DoubleRow was
tried and rejected: e4m3's ~3.6% element noise propagates at full strength
through the softmax-weighted mean (no sqrt-N averaging) and blows the 2e-2
budget. fp16 keeps the error at ~1e-3. DMA also halves (x ships fp16, out
partials return fp16).

Engine budget: ACT does only the exps (the critical 33.5M-element
transcendental load); q/k/v PSUM->SBUF conversions are fused scale+bias
tensor_scalar ops on DVE; out-proj PSUM drains alternate DVE/ACT.
"""
import os
import sys

sys.path.insert(0, "/opt/trn_rl_repo")
import numpy as np

B, S, D, H = 4, 2048, 2048, 16
HD = 128
NCORES = 8
HP = H // NCORES          # heads per core = 2
DC = HP * HD              # per-core slice of D = 256
TOK = B * S               # 8192
SCALE = HD ** -0.5
NDC = D // 128            # 16 contraction chunks for the projections
SPAN = 256                # token span per projection step
NSPAN = S // SPAN         # 8 spans per batch
QS = 512                  # query span in attention
NQS = S // QS             # 4
NKC = S // 128            # 16 key chunks

LAST_EXEC_NS = None
_BUILT = None


def _chunk128(w: np.ndarray) -> np.ndarray:
    """[D, N] -> [128, NDC, N]: contraction row 128*c + p."""
    n = w.shape[1]
    return np.ascontiguousarray(
        w.reshape(NDC, 128, n).transpose(1, 0, 2))


def _build():
    global _BUILT
    if _BUILT is not None:
        return _BUILT
    import concourse.tile as tile
    from concourse import bacc, mybir

    F16 = mybir.dt.float16
    F32 = mybir.dt.float32
    Exp = mybir.ActivationFunctionType.Exp
    Mult = mybir.AluOpType.mult
    Add = mybir.AluOpType.add

    nc = bacc.Bacc("TRN2", target_bir_lowering=False, debug=False)
    xt = nc.dram_tensor("xt", [128, NDC, TOK], F16, kind="ExternalInput")
    wq = nc.dram_tensor("wq", [128, NDC, DC], F16, kind="ExternalInput")
    wk = nc.dram_tensor("wk", [128, NDC, DC], F16, kind="ExternalInput")
    wv = nc.dram_tensor("wv", [128, NDC, DC], F16, kind="ExternalInput")
    wo = nc.dram_tensor("wo", [DC, D], F16, kind="ExternalInput")
    bq2 = nc.dram_tensor("bq2", [HD, HP], F32, kind="ExternalInput")
    bk2 = nc.dram_tensor("bk2", [HD, HP], F32, kind="ExternalInput")
    out = nc.dram_tensor("out", [TOK, D], F16, kind="ExternalOutput")

    with tile.TileContext(nc) as tc:
        with tc.tile_pool(name="const", bufs=1) as cpool, \
             tc.tile_pool(name="xp", bufs=3) as xpool, \
             tc.tile_pool(name="bt", bufs=1) as bpool, \
             tc.tile_pool(name="at", bufs=3) as apool, \
             tc.tile_pool(name="ot", bufs=2) as opool, \
             tc.tile_pool(name="ps", bufs=1, space="PSUM") as ps:

            wq_sb = cpool.tile([128, NDC, DC], F16)
            wk_sb = cpool.tile([128, NDC, DC], F16)
            wv_sb = cpool.tile([128, NDC, DC], F16)
            wo_sb = cpool.tile([128, HP, D], F16)
            ones_sb = cpool.tile([128, 128], F16)
            ebias_sb = cpool.tile([128, 1], F32)
            bq_sb = cpool.tile([HD, HP], F32)
            bk_sb = cpool.tile([HD, HP], F32)
            # wq arrives in two halves so the first Q-projection chunks can
            # start after 512KB instead of 1MB
            nc.sync.dma_start(out=wq_sb[:, 0:NDC // 2, :],
                              in_=wq[:, 0:NDC // 2, :])
            nc.sync.dma_start(out=wq_sb[:, NDC // 2:NDC, :],
                              in_=wq[:, NDC // 2:NDC, :])
            nc.sync.dma_start(out=bq_sb, in_=bq2[:, :])
            nc.sync.dma_start(out=bk_sb, in_=bk2[:, :])
            nc.vector.memset(ones_sb, 1.0)
            nc.vector.memset(ebias_sb, 0.0)

            prefetched = {}
            for b in range(B):
                # ---- A) Q/K/V projections for batch b ----
                qt_b = bpool.tile([128, HP, S], F16, name="qt_b", tag="qt_b")
                kt_b = bpool.tile([128, HP, S], F16, name="kt_b", tag="kt_b")
                v_b = bpool.tile([128, NKC, DC], F16, name="v_b", tag="v_b")
                for sp in range(NSPAN):
                    t0 = b * S + sp * SPAN
                    if (b, sp) in prefetched:
                        xsp = prefetched.pop((b, sp))
                    else:
                        xsp = xpool.tile([128, NDC, SPAN], F16, name="xsp",
                                         tag="xsp")
                        if b == 0 and sp == 0:
                            # first span arrives in chunk quarters so the
                            # very first matmuls are not gated on the full
                            # 1MB span
                            for c4 in range(0, NDC, 4):
                                nc.sync.dma_start(
                                    out=xsp[:, c4:c4 + 4, :],
                                    in_=xt[:, c4:c4 + 4, t0:t0 + SPAN])
                        else:
                            nc.sync.dma_start(out=xsp,
                                              in_=xt[:, :, t0:t0 + SPAN])
                    if b == 0 and sp == 0:
                        # wk/wv queue behind wq + the first x span so the PE
                        # can start the Q projection as early as possible
                        nc.sync.dma_start(out=wk_sb, in_=wk[:, :, :])
                        nc.sync.dma_start(out=wv_sb, in_=wv[:, :, :])
                    for h in range(HP):
                        # Q and K accumulate into halves of one PSUM bank
                        qkps = ps.tile([128, 2 * SPAN], F32, name="qkps",
                                       tag="pj", bufs=2)
                        for c in range(NDC):
                            nc.tensor.matmul(
                                qkps[:, 0:SPAN],
                                wq_sb[:, c, h * HD:(h + 1) * HD],
                                xsp[:, c, :],
                                start=(c == 0), stop=(c == NDC - 1))
                        for c in range(NDC):
                            nc.tensor.matmul(
                                qkps[:, SPAN:2 * SPAN],
                                wk_sb[:, c, h * HD:(h + 1) * HD],
                                xsp[:, c, :],
                                start=(c == 0), stop=(c == NDC - 1))
                        nc.vector.tensor_scalar(
                            qt_b[:, h, sp * SPAN:(sp + 1) * SPAN],
                            qkps[:, 0:SPAN], 1.0, bq_sb[:, h:h + 1],
                            Mult, Add)
                        nc.vector.tensor_scalar(
                            kt_b[:, h, sp * SPAN:(sp + 1) * SPAN],
                            qkps[:, SPAN:2 * SPAN], 1.0,
                            bk_sb[:, h:h + 1], Mult, Add)
                    # both V token-chunks accumulate into one PSUM bank
                    vps = ps.tile([128, 2 * DC], F32, name="vps", tag="pj",
                                  bufs=2)
                    for tch in range(SPAN // 128):
                        for c in range(NDC):
                            nc.tensor.matmul(
                                vps[:, tch * DC:(tch + 1) * DC],
                                xsp[:, c, tch * 128:(tch + 1) * 128],
                                wv_sb[:, c, :],
                                start=(c == 0), stop=(c == NDC - 1))
                    for tch in range(SPAN // 128):
                        nc.vector.tensor_copy(
                            v_b[:, sp * (SPAN // 128) + tch, :],
                            vps[:, tch * DC:(tch + 1) * DC])

                if b == 0:
                    # deferred so batch-0 x spans win the DMA queue at startup
                    nc.sync.dma_start(
                        out=wo_sb, in_=wo.rearrange("(c p) n -> p c n", p=128))
                if b + 1 < B:
                    # prefetch the next batch's first two x spans now, ahead
                    # of this batch's 8MB of output DMAs in the ring, so the
                    # next projections don't stall at the batch boundary
                    for psp in range(2):
                        pt0 = (b + 1) * S + psp * SPAN
                        pxsp = xpool.tile([128, NDC, SPAN], F16, name="xsp",
                                          tag="xsp")
                        nc.sync.dma_start(out=pxsp,
                                          in_=xt[:, :, pt0:pt0 + SPAN])
                        prefetched[(b + 1, psp)] = pxsp

                # ---- B) attention + interleaved partial out-projection ----
                avt_b = bpool.tile([128, HP, S], F16, name="avt_b",
                                   tag="avt_b")

                def emit_outproj(qs, b=b, avt_b=avt_b):
                    # partial out-projection for query span qs; deferred
                    # until the next span's first head has issued so the
                    # avt(h1) normalize sits well behind ~10us of PE work.
                    # (b/avt_b bound at def time: the final span's call runs
                    # inside the NEXT batch's projection stream)
                    for tloc in range(QS // 128):
                        tch = qs * (QS // 128) + tloc
                        out_sb = opool.tile([128, D], F16, name="out_sb",
                                            tag="out_sb")
                        for dsp in range(D // 512):
                            ops = ps.tile([128, 512], F32, name="ops",
                                          tag="pj", bufs=2)
                            for h in range(HP):
                                nc.tensor.matmul(
                                    ops,
                                    avt_b[:, h, tch * 128:(tch + 1) * 128],
                                    wo_sb[:, h, dsp * 512:(dsp + 1) * 512],
                                    start=(h == 0), stop=(h == HP - 1))
                            # split the PSUM drain across DVE and ACT so
                            # neither engine gates the PE
                            if dsp % 2 == 0:
                                nc.vector.tensor_copy(
                                    out_sb[:, dsp * 512:(dsp + 1) * 512], ops)
                            else:
                                nc.scalar.copy(
                                    out_sb[:, dsp * 512:(dsp + 1) * 512], ops)
                            if b == B - 1 and tch == S // 128 - 1:
                                # last tile: drain per 512-col slice so the
                                # final DMA isn't serialized behind all four
                                # copies
                                nc.sync.dma_start(
                                    out=out[b * S + tch * 128:
                                            b * S + (tch + 1) * 128,
                                            dsp * 512:(dsp + 1) * 512],
                                    in_=out_sb[:, dsp * 512:(dsp + 1) * 512])
                        if not (b == B - 1 and tch == S // 128 - 1):
                            nc.sync.dma_start(
                                out=out[b * S + tch * 128:
                                        b * S + (tch + 1) * 128, :],
                                in_=out_sb)

                for qs in range(NQS):
                    for h in range(HP):
                        q_sl = qt_b[:, h, qs * QS:(qs + 1) * QS]
                        av_ps = ps.tile([HD, QS], F32, name="av_ps",
                                        tag="acc", bufs=2)
                        dn_ps = ps.tile([128, QS], F32, name="dn_ps",
                                        tag="acc", bufs=2)

                        def emit_av(kp, p_prev):
                            # AV and the softmax-denominator ones-matmul both
                            # consume the exp tile on the PE — keeps the PE
                            # dense (no DVE/GPSIMD reduction chains). dn goes
                            # first so its stop lands earlier and the DVE
                            # reciprocal overlaps the AV tail.
                            for j in range(2):
                                kc = 2 * kp + j
                                nc.tensor.matmul(
                                    dn_ps, ones_sb, p_prev[:, j, :],
                                    start=(kc == 0), stop=(kc == NKC - 1))
                            for j in range(2):
                                kc = 2 * kp + j
                                nc.tensor.matmul(
                                    av_ps, v_b[:, kc, h * HD:(h + 1) * HD],
                                    p_prev[:, j, :],
                                    start=(kc == 0), stop=(kc == NKC - 1))

                        p_prev = None
                        for kp in range(NKC // 2):
                            # two key-chunks share one psum tile and one exp;
                            # AV of pair kp-1 is emitted after the scores of
                            # pair kp so the PE never heads-of-line blocks on
                            # the exp it needs
                            s_ps = ps.tile([128, 2, QS], F32, name="s_ps",
                                           tag="s", bufs=2)
                            p_sb = apool.tile([128, 2, QS], F16, name="p_sb",
                                              tag="p", bufs=3)
                            for j in range(2):
                                kc = 2 * kp + j
                                nc.tensor.matmul(
                                    s_ps[:, j, :],
                                    kt_b[:, h, kc * 128:(kc + 1) * 128], q_sl,
                                    start=True, stop=True)
                            nc.scalar.activation(
                                p_sb, s_ps, Exp, scale=SCALE,
                                bias=ebias_sb[:, 0:1])
                            if p_prev is not None:
                                emit_av(kp - 1, p_prev)
                            p_prev = p_sb
                        emit_av(NKC // 2 - 1, p_prev)
                        recip = apool.tile([128, QS], F32, name="recip",
                                           tag="recip", bufs=1)
                        nc.vector.reciprocal_approx_fast(recip, dn_ps)
                        nc.vector.tensor_mul(
                            avt_b[:, h, qs * QS:(qs + 1) * QS], av_ps, recip)
                        if h == 0 and qs > 0:
                            emit_outproj(qs - 1)
                emit_outproj(NQS - 1)
    nc.compile()
    _BUILT = nc
    return nc


def _install_trace_hooks():
    import types
    try:
        import antenv.axon_hooks  # noqa: F401
        return True
    except ImportError:
        pass
    try:
        from trn_agent_boot.trn_boot import _ntff_profile_via_ctypes
        hook = _ntff_profile_via_ctypes('/opt/axon/libaxon_pjrt.so')
        if hook is None:
            return False
        m = types.ModuleType('antenv.axon_hooks')
        m.get_axon_ntff_profile_hook = lambda: hook
        sys.modules['antenv.axon_hooks'] = m
        from concourse import bass_utils
        bass_utils.upload_artifacts = lambda tmpdir: "local://" + tmpdir
        return True
    except Exception:
        return False


def kernel(x, wq, bq, wk, bk, wv, bv, wo, bo):
    global LAST_EXEC_NS
    from concourse.bass_utils import run_bass_kernel_spmd

    x = np.asarray(x, dtype=np.float32)
    wq = np.asarray(wq, dtype=np.float32)
    bq = np.asarray(bq, dtype=np.float32)
    wk = np.asarray(wk, dtype=np.float32)
    bk = np.asarray(bk, dtype=np.float32)
    wv = np.asarray(wv, dtype=np.float32)
    bv = np.asarray(bv, dtype=np.float32)
    wo = np.asarray(wo, dtype=np.float32)
    bo = np.asarray(bo, dtype=np.float32)

    xt16 = _chunk128(x.reshape(TOK, D).T.astype(np.float16))
    in_maps = []
    for i in range(NCORES):
        sl = slice(i * DC, (i + 1) * DC)
        in_maps.append({
            "xt": xt16,
            "wq": _chunk128(wq[:, sl].astype(np.float16)),
            "wk": _chunk128(wk[:, sl].astype(np.float16)),
            "wv": _chunk128(wv[:, sl].astype(np.float16)),
            "wo": wo[sl, :].astype(np.float16),
            "bq2": np.ascontiguousarray(bq[sl].reshape(HP, HD).T),
            "bk2": np.ascontiguousarray(bk[sl].reshape(HP, HD).T),
        })

    trace = bool(os.environ.get("KERNEL_TRACE"))
    if trace:
        trace = _install_trace_hooks()

    nc = _build()
    res = run_bass_kernel_spmd(nc, in_maps, list(range(NCORES)), trace=trace)
    LAST_EXEC_NS = res.exec_time_ns

    total = np.zeros((TOK, D), dtype=np.float32)
    for r in res.results:
        total += r["out"].astype(np.float32)
    # V-bias folds into a constant row: softmax rows sum to 1, so
    # attention(V + 1*bv^T) = attention(V) + 1*bv^T, and (bv @ wo) adds to bo.
    total += bo + bv @ wo
    return total.reshape(B, S, D)


# revision 23
# speedup vs baseline: 1.0206x; 1.0203x over previous
"""Multi-head attention (B=4, S=2048, D=2048, H=16) on 8 trn2 NeuronCores.

Sharding: tensor-parallel over heads — 2 heads per core. Each core computes
its heads' Q/K/V projections, full attention for those heads, and a partial
output projection (its 256 rows of wo). The host sums the 8 partial outputs.

Precision strategy: fp16 end to end (x, weights, q/k/v, exp, avt, wo all
fp16; PSUM accumulation fp32). fp16 streams at the same 1 col/cycle as
fp32r, but the stationary-operand loads get FWL (fast weight load — 2
elements per 32-bit read, compiler-automatic for non-fp32 dtypes), which
fp32r cannot use; that removes the serialized 4-byte weight-load overhead
that dominated the fp32r baseline's non-stream PE time. fp8 DoubleRow was
tried and rejected: e4m3's ~3.6% element noise propagates at full strength
through the softmax-weighted mean (no sqrt-N averaging) and blows the 2e-2
budget. fp16 keeps the error at ~1e-3. DMA also halves (x ships fp16, out
partials return fp16).

Engine budget: ACT does only the exps (the critical 33.5M-element
transcendental load); q/k/v PSUM->SBUF conversions are fused scale+bias
tensor_scalar ops on DVE; out-proj PSUM drains alternate DVE/ACT.
"""
import os
import sys

sys.path.insert(0, "/opt/trn_rl_repo")
import numpy as np

B, S, D, H = 4, 2048, 2048, 16
HD = 128
NCORES = 8
HP = H // NCORES          # heads per core = 2
DC = HP * HD              # per-core slice of D = 256
TOK = B * S               # 8192
SCALE = HD ** -0.5
NDC = D // 128            # 16 contraction chunks for the projections
SPAN = 256                # token span per projection step
NSPAN = S // SPAN         # 8 spans per batch
QS = 512                  # query span in attention
NQS = S // QS             # 4
NKC = S // 128            # 16 key chunks

LAST_EXEC_NS = None
_BUILT = None


def _chunk128(w: np.ndarray) -> np.ndarray:
    """[D, N] -> [128, NDC, N]: contraction row 128*c + p."""
    n = w.shape[1]
    return np.ascontiguousarray(
        w.reshape(NDC, 128, n).transpose(1, 0, 2))


def _build():
    global _BUILT
    if _BUILT is not None:
        return _BUILT
    import concourse.tile as tile
    from concourse import bacc, mybir

    F16 = mybir.dt.float16
    F32 = mybir.dt.float32
    Exp = mybir.ActivationFunctionType.Exp
    Mult = mybir.AluOpType.mult
    Add = mybir.AluOpType.add

    nc = bacc.Bacc("TRN2", target_bir_lowering=False, debug=False)
    xt = nc.dram_tensor("xt", [128, NDC, TOK], F16, kind="ExternalInput")
    wq = nc.dram_tensor("wq", [128, NDC, DC], F16, kind="ExternalInput")
    wk = nc.dram_tensor("wk", [128, NDC, DC], F16, kind="ExternalInput")
    wv = nc.dram_tensor("wv", [128, NDC, DC], F16, kind="ExternalInput")
    wo = nc.dram_tensor("wo", [DC, D], F16, kind="ExternalInput")
    bq2 = nc.dram_tensor("bq2", [HD, HP], F32, kind="ExternalInput")
    bk2 = nc.dram_tensor("bk2", [HD, HP], F32, kind="ExternalInput")
    out = nc.dram_tensor("out", [TOK, D], F16, kind="ExternalOutput")

    with tile.TileContext(nc) as tc:
        with tc.tile_pool(name="const", bufs=1) as cpool, \
             tc.tile_pool(name="xp", bufs=3) as xpool, \
             tc.tile_pool(name="bt", bufs=1) as bpool, \
             tc.tile_pool(name="at", bufs=3) as apool, \
             tc.tile_pool(name="ot", bufs=2) as opool, \
             tc.tile_pool(name="ps", bufs=1, space="PSUM") as ps:

            wq_sb = cpool.tile([128, NDC, DC], F16)
            wk_sb = cpool.tile([128, NDC, DC], F16)
            wv_sb = cpool.tile([128, NDC, DC], F16)
            wo_sb = cpool.tile([128, HP, D], F16)
            ones_sb = cpool.tile([128, 128], F16)
            ebias_sb = cpool.tile([128, 1], F32)
            bq_sb = cpool.tile([HD, HP], F32)
            bk_sb = cpool.tile([HD, HP], F32)
            # wq arrives in two halves so the first Q-projection chunks can
            # start after 512KB instead of 1MB
            nc.sync.dma_start(out=wq_sb[:, 0:NDC // 2, :],
                              in_=wq[:, 0:NDC // 2, :])
            nc.sync.dma_start(out=wq_sb[:, NDC // 2:NDC, :],
                              in_=wq[:, NDC // 2:NDC, :])
            nc.sync.dma_start(out=bq_sb, in_=bq2[:, :])
            nc.sync.dma_start(out=bk_sb, in_=bk2[:, :])
            nc.vector.memset(ones_sb, 1.0)
            nc.vector.memset(ebias_sb, 0.0)

            prefetched = {}
            for b in range(B):
                # ---- A) Q/K/V projections for batch b ----
                qt_b = bpool.tile([128, HP, S], F16, name="qt_b", tag="qt_b")
                kt_b = bpool.tile([128, HP, S], F16, name="kt_b", tag="kt_b")
                v_b = bpool.tile([128, NKC, DC], F16, name="v_b", tag="v_b")
                for sp in range(NSPAN):
                    t0 = b * S + sp * SPAN
                    if (b, sp) in prefetched:
                        xsp = prefetched.pop((b, sp))
                    else:
                        xsp = xpool.tile([128, NDC, SPAN], F16, name="xsp",
                                         tag="xsp")
                        if b == 0 and sp == 0:
                            # first span arrives in chunk quarters so the
                            # very first matmuls are not gated on the full
                            # 1MB span
                            for c4 in range(0, NDC, 4):
                                nc.sync.dma_start(
                                    out=xsp[:, c4:c4 + 4, :],
                                    in_=xt[:, c4:c4 + 4, t0:t0 + SPAN])
                        else:
                            nc.sync.dma_start(out=xsp,
                                              in_=xt[:, :, t0:t0 + SPAN])
                    if b == 0 and sp == 0:
                        # wk/wv queue behind wq + the first x span so the PE
                        # can start the Q projection as early as possible
                        nc.sync.dma_start(out=wk_sb, in_=wk[:, :, :])
                        nc.sync.dma_start(out=wv_sb, in_=wv[:, :, :])
                    for h in range(HP):
                        # Q and K accumulate into halves of one PSUM bank
                        qkps = ps.tile([128, 2 * SPAN], F32, name="qkps",
                                       tag="pj", bufs=2)
                        for c in range(NDC):
                            nc.tensor.matmul(
                                qkps[:, 0:SPAN],
                                wq_sb[:, c, h * HD:(h + 1) * HD],
                                xsp[:, c, :],
                                start=(c == 0), stop=(c == NDC - 1))
                        for c in range(NDC):
                            nc.tensor.matmul(
                                qkps[:, SPAN:2 * SPAN],
                                wk_sb[:, c, h * HD:(h + 1) * HD],
                                xsp[:, c, :],
                                start=(c == 0), stop=(c == NDC - 1))
                        nc.vector.tensor_scalar(
                            qt_b[:, h, sp * SPAN:(sp + 1) * SPAN],
                            qkps[:, 0:SPAN], 1.0, bq_sb[:, h:h + 1],
                            Mult, Add)
                        nc.vector.tensor_scalar(
                            kt_b[:, h, sp * SPAN:(sp + 1) * SPAN],
                            qkps[:, SPAN:2 * SPAN], 1.0,
                            bk_sb[:, h:h + 1], Mult, Add)
                    # both V token-chunks accumulate into one PSUM bank
                    vps = ps.tile([128, 2 * DC], F32, name="vps", tag="pj",
                                  bufs=2)
                    for tch in range(SPAN // 128):
                        for c in range(NDC):
                            nc.tensor.matmul(
                                vps[:, tch * DC:(tch + 1) * DC],
                                xsp[:, c, tch * 128:(tch + 1) * 128],
                                wv_sb[:, c, :],
                                start=(c == 0), stop=(c == NDC - 1))
                    for tch in range(SPAN // 128):
                        nc.vector.tensor_copy(
                            v_b[:, sp * (SPAN // 128) + tch, :],
                            vps[:, tch * DC:(tch + 1) * DC])

                if b == 0:
                    # deferred so batch-0 x spans win the DMA queue at startup
                    nc.sync.dma_start(
                        out=wo_sb, in_=wo.rearrange("(c p) n -> p c n", p=128))
                if b + 1 < B:
                    # prefetch the next batch's first two x spans now, ahead
                    # of this batch's 8MB of output DMAs in the ring, so the
                    # next projections don't stall at the batch boundary
                    for psp in range(2):
                        pt0 = (b + 1) * S + psp * SPAN
                        pxsp = xpool.tile([128, NDC, SPAN], F16, name="xsp",
                                          tag="xsp")
                        nc.sync.dma_start(out=pxsp,
                                          in_=xt[:, :, pt0:pt0 + SPAN])
                        prefetched[(b + 1, psp)] = pxsp

                # ---- B) attention + interleaved partial out-projection ----
                avt_b = bpool.tile([128, HP, S], F16, name="avt_b",
                                   tag="avt_b")

                def emit_outproj(qs, b=b, avt_b=avt_b):
                    # partial out-projection for query span qs; deferred
                    # until the next span's first head has issued so the
                    # avt(h1) normalize sits well behind ~10us of PE work.
                    # (b/avt_b bound at def time: the final span's call runs
                    # inside the NEXT batch's projection stream)
                    for tloc in range(QS // 128):
                        tch = qs * (QS // 128) + tloc
                        out_sb = opool.tile([128, D], F16, name="out_sb",
                                            tag="out_sb")
                        for dsp in range(D // 512):
                            ops = ps.tile([128, 512], F32, name="ops",
                                          tag="pj", bufs=2)
                            for h in range(HP):
                                nc.tensor.matmul(
                                    ops,
                                    avt_b[:, h, tch * 128:(tch + 1) * 128],
                                    wo_sb[:, h, dsp * 512:(dsp + 1) * 512],
                                    start=(h == 0), stop=(h == HP - 1))
                            # split the PSUM drain across DVE and ACT so
                            # neither engine gates the PE
                            if dsp % 2 == 0:
                                nc.vector.tensor_copy(
                                    out_sb[:, dsp * 512:(dsp + 1) * 512], ops)
                            else:
                                nc.scalar.copy(
                                    out_sb[:, dsp * 512:(dsp + 1) * 512], ops)
                            if b == B - 1 and tch == S // 128 - 1:
                                # last tile: drain per 512-col slice so the
                                # final DMA isn't serialized behind all four
                                # copies
                                nc.sync.dma_start(
                                    out=out[b * S + tch * 128:
                                            b * S + (tch + 1) * 128,
                                            dsp * 512:(dsp + 1) * 512],
                                    in_=out_sb[:, dsp * 512:(dsp + 1) * 512])
                        if not (b == B - 1 and tch == S // 128 - 1):
                            nc.sync.dma_start(
                                out=out[b * S + tch * 128:
                                        b * S + (tch + 1) * 128, :],
                                in_=out_sb)

                for qs in range(NQS):
                    for h in range(HP):
                        q_sl = qt_b[:, h, qs * QS:(qs + 1) * QS]
                        av_ps = ps.tile([HD, QS], F32, name="av_ps",
                                        tag="acc", bufs=2)
                        dn_ps = ps.tile([128, QS], F32, name="dn_ps",
                                        tag="acc", bufs=2)

                        def emit_av(kc, p_kc):
                            # AV and the softmax-denominator ones-matmul both
                            # consume the exp tile on the PE — keeps the PE
                            # dense (no DVE/GPSIMD reduction chains). dn goes
                            # first so its stop lands earlier and the DVE
                            # reciprocal overlaps the AV tail.
                            nc.tensor.matmul(
                                dn_ps, ones_sb, p_kc,
                                start=(kc == 0), stop=(kc == NKC - 1))
                            nc.tensor.matmul(
                                av_ps, v_b[:, kc, h * HD:(h + 1) * HD],
                                p_kc, start=(kc == 0), stop=(kc == NKC - 1))

                        pq = []
                        for kc in range(NKC):
                            # per-chunk exp: the PE's wait on the first exp
                            # of a span is one 512-wide activation (~0.7us),
                            # and AV trails the scores by two chunks so the
                            # ACT engine always has a two-deep cushion
                            s_ps = ps.tile([128, QS], F32, name="s_ps",
                                           tag="s", bufs=4)
                            p_sb = apool.tile([128, QS], F16, name="p_sb",
                                              tag="p", bufs=6)
                            nc.tensor.matmul(
                                s_ps,
                                kt_b[:, h, kc * 128:(kc + 1) * 128], q_sl,
                                start=True, stop=True)
                            nc.scalar.activation(
                                p_sb, s_ps, Exp, scale=SCALE,
                                bias=ebias_sb[:, 0:1])
                            pq.append(p_sb)
                            if len(pq) > 2:
                                emit_av(kc - 2, pq.pop(0))
                        for i, p_sb in enumerate(pq):
                            emit_av(NKC - len(pq) + i, p_sb)
                        recip = apool.tile([128, QS], F32, name="recip",
                                           tag="recip", bufs=1)
                        nc.vector.reciprocal_approx_fast(recip, dn_ps)
                        nc.vector.tensor_mul(
                            avt_b[:, h, qs * QS:(qs + 1) * QS], av_ps, recip)
                        if h == 0 and qs > 0:
                            emit_outproj(qs - 1)
                emit_outproj(NQS - 1)
    nc.compile()
    _BUILT = nc
    return nc


def _install_trace_hooks():
    import types
    try:
        import antenv.axon_hooks  # noqa: F401
        return True
    except ImportError:
        pass
    try:
        from trn_agent_boot.trn_boot import _ntff_profile_via_ctypes
        hook = _ntff_profile_via_ctypes('/opt/axon/libaxon_pjrt.so')
        if hook is None:
            return False
        m = types.ModuleType('antenv.axon_hooks')
        m.get_axon_ntff_profile_hook = lambda: hook
        sys.modules['antenv.axon_hooks'] = m
        from concourse import bass_utils
        bass_utils.upload_artifacts = lambda tmpdir: "local://" + tmpdir
        return True
    except Exception:
        return False


def kernel(x, wq, bq, wk, bk, wv, bv, wo, bo):
    global LAST_EXEC_NS
    from concourse.bass_utils import run_bass_kernel_spmd

    x = np.asarray(x, dtype=np.float32)
    wq = np.asarray(wq, dtype=np.float32)
    bq = np.asarray(bq, dtype=np.float32)
    wk = np.asarray(wk, dtype=np.float32)
    bk = np.asarray(bk, dtype=np.float32)
    wv = np.asarray(wv, dtype=np.float32)
    bv = np.asarray(bv, dtype=np.float32)
    wo = np.asarray(wo, dtype=np.float32)
    bo = np.asarray(bo, dtype=np.float32)

    xt16 = _chunk128(x.reshape(TOK, D).T.astype(np.float16))
    in_maps = []
    for i in range(NCORES):
        sl = slice(i * DC, (i + 1) * DC)
        in_maps.append({
            "xt": xt16,
            "wq": _chunk128(wq[:, sl].astype(np.float16)),
            "wk": _chunk128(wk[:, sl].astype(np.float16)),
            "wv": _chunk128(wv[:, sl].astype(np.float16)),
            "wo": wo[sl, :].astype(np.float16),
            "bq2": np.ascontiguousarray(bq[sl].reshape(HP, HD).T),
            "bk2": np.ascontiguousarray(bk[sl].reshape(HP, HD).T),
        })

    trace = bool(os.environ.get("KERNEL_TRACE"))
    if trace:
        trace = _install_trace_hooks()

    nc = _build()
    res = run_bass_kernel_spmd(nc, in_maps, list(range(NCORES)), trace=trace)
    LAST_EXEC_NS = res.exec_time_ns

    total = np.zeros((TOK, D), dtype=np.float32)
    for r in res.results:
        total += r["out"].astype(np.float32)
    # V-bias folds into a constant row: softmax rows sum to 1, so
    # attention(V + 1*bv^T) = attention(V) + 1*bv^T, and (bv @ wo) adds to bo.
    total += bo + bv @ wo
    return total.reshape(B, S, D)
